# revision 13
# baseline (speedup 1.0000x reference)
"""BinaryLinear Trainium2 kernel: y = x @ sign(W).T + bias.

Full shapes: x [8192, 2048] f32, W [2048, 2048] f32, bias [2048] f32.
Strategy: data-parallel over 8 NeuronCores — shard x rows (1024/core),
replicate W and bias, no collectives. Host only shards / lays out /
down-casts to the kernel's compute precisions; all math (sign binarize,
matmul, bias add) runs on device.

Numerics: mixed precision on the contraction. The first KB16=1024 of
K=2048 runs in bf16; the last KF8=1024 runs in fp8-e4m3 with the
tensor engine's DoubleRow perf mode (2x matmul throughput). W is
binarized on-device to {-0.5, +0.5}: the bf16 section in place from
bf16 weights; the fp8 section ships as raw fp8 bytes and the device
extracts the sign from the byte MSB ((byte < 128) - 0.5), which is
exact even where the fp8 cast flushed a tiny weight to +-0. The factor
2 is folded into the PSUM eviction (out = 2*psum + bias, per-partition
bias AP). Host-side sim of this exact scheme measures scale-relative
absmax err 0.01939 vs the f32 reference (gate 2e-2), and the device
reproduces the sim bit-for-bit at that precision; accumulation is f32
in PSUM throughout.

Schedule: the GEMM computes y^T with W *stationary* and x *moving*,
k-outer accumulation over 256-out-col strips, 4 PSUM banks per strip
double-buffered across strips. Consecutive matmuls alternate PSUM
banks, which lets the PE pipeline the stationary reload under the
moving stream (measured cadence 216ns/matmul ~ the 213ns roofline at
the PE's full 2.4GHz p-state; warmup matmuls ramp the clock while the
first tiles stream in). Input DMAs are issued in exact consumption
order as ~0.25-0.5 MiB pieces alternating between the sync and scalar
DGE queues — descriptor generation costs ~650ns per dma_start per
queue, and each queue drains roughly in order, so the critical early
tiles must not queue behind bulk W transfers. W streams
strip-by-strip so the PE never waits on the full-W DMA; binarize ops
are range-aligned with the DMA pieces. The last strip is processed as
two 128-col halves so its first eviction overlaps the second half's
matmuls (shrinks the end-of-kernel drain). Output is y^T in bf16
(halves the out traffic); host transposes/upcasts after the gather.
"""

import numpy as np
import ml_dtypes

N_CORES = 8
N_ROWS = 8192
D_IN = 2048
D_OUT = 2048
N_SH = N_ROWS // N_CORES      # 1024 x-rows per core

KB = 128                      # contraction block (SBUF partitions)
KF8 = 1024                    # trailing K columns computed in fp8 DoubleRow
KB16 = D_IN - KF8             # leading K columns computed in bf16
NKB = KB16 // KB              # bf16 k-tiles (8)
NPR = KF8 // (2 * KB)         # fp8 DoubleRow k-tile pairs (4)
NSUB = NKB + 2 * NPR          # k-subtiles per strip tile (16)
NSTRIP = 8                    # output strips
SCOL = D_OUT // NSTRIP        # 256 out-cols per strip (2 n128 blocks)
MH = 512                      # moving half-width (PSUM bank = 512 f32)

_cache = {}


def build_nc(nsh=N_SH, dout=D_OUT, warmup_mms=7):
    import concourse.bass as bass
    import concourse.bacc as bacc
    import concourse.tile as tile
    from concourse import mybir

    f32 = mybir.dt.float32
    bf16 = mybir.dt.bfloat16
    fp8 = mybir.dt.float8e4

    nmh = nsh // MH           # 2 moving halves
    ng = dout // KB           # 16 n128 blocks total

    nc = bacc.Bacc("TRN2", debug=False)
    xtb = nc.dram_tensor("xtb", [KB, NKB, nsh], bf16, kind="ExternalInput").ap()
    xt8 = nc.dram_tensor("xt8", [KB, 2 * NPR, nsh], fp8, kind="ExternalInput").ap()
    wt = nc.dram_tensor("wt", [NSTRIP, KB, NKB, SCOL], bf16, kind="ExternalInput").ap()
    wt8 = nc.dram_tensor("wt8", [NSTRIP, KB, 2 * NPR, SCOL], mybir.dt.uint8, kind="ExternalInput").ap()
    biasr = nc.dram_tensor("biasr", [KB, ng], f32, kind="ExternalInput").ap()
    yt = nc.dram_tensor("yt", [dout, nsh], bf16, kind="ExternalOutput").ap()

    with tile.TileContext(nc) as tc:
        with (
            tc.tile_pool(name="wb", bufs=1) as wb_pool,
            tc.tile_pool(name="xb", bufs=1) as xb_pool,
            tc.tile_pool(name="misc", bufs=1) as misc_pool,
            tc.tile_pool(name="out", bufs=4) as out_pool,
            tc.tile_pool(name="psum", bufs=8, space=bass.MemorySpace.PSUM) as psum_pool,
        ):
            # PE clock-gate warmup on a zeroed scratch tile; ramps the PE
            # p-state while the first strip's W and the x tiles stream in.
            if warmup_mms:
                dummy = misc_pool.tile([128, MH], bf16, tag="dummy")
                # memset on GpSimd: its preamble finishes ~1us before the
                # DVE's, so the warmup matmuls can start that much earlier
                nc.gpsimd.memset(dummy[:, :], 0.0)
                wps = psum_pool.tile([128, MH], f32, tag="ps", name="ps_warm")
                for _ in range(warmup_mms):
                    nc.tensor.matmul(
                        wps[:, :], dummy[:, 0:KB], dummy[:, :],
                        start=True, stop=True,
                    )

            # ---- tiles ----
            ws = []        # per-strip bf16-section W tile [128, NKB, SCOL]
            w8s = []       # per-strip fp8-section W bytes (shipped fp8, viewed u8)
            w8 = []        # per-strip binarized fp8 W [128, 2*NPR, SCOL]
            for s in range(NSTRIP):
                ws.append(wb_pool.tile([KB, NKB, SCOL], bf16, tag=f"ws{s}", name=f"ws{s}"))
                w8s.append(wb_pool.tile([KB, 2 * NPR, SCOL], mybir.dt.uint8, tag=f"w8s{s}", name=f"w8s{s}"))
                w8.append(wb_pool.tile([KB, 2 * NPR, SCOL], fp8, tag=f"w8{s}", name=f"w8{s}"))
            xb = []        # per-k x tiles [128, nsh] bf16
            for k in range(NKB):
                xb.append(xb_pool.tile([KB, nsh], bf16, tag=f"xb{k}", name=f"xb{k}"))
            x8 = xb_pool.tile([KB, 2 * NPR, nsh], fp8, tag="x8", name="x8")
            bias_sb = misc_pool.tile([KB, ng], f32, tag="bias")

            def xslice(k, mh):
                return xb[k][:, mh * MH:(mh + 1) * MH]

            # ---- input DMAs ----
            # Pieces sized ~0.25-0.5 MiB, issued in exact consumption order,
            # alternating between the two DGE queues: descriptor generation
            # costs ~650ns per dma_start per queue, and transfers on a queue
            # drain roughly in order, so the critical early tiles must not
            # queue behind bulk W transfers.
            def wpiece(s, a, b):
                return (ws[s][:, a:b, :], wt[s, :, a:b, :])

            def xpiece(k):
                return (xb[k][:, :], xtb[:, k, :])

            w0cuts = [0, 2, 5, NKB]
            w0p = [wpiece(0, a, b) for a, b in zip(w0cuts, w0cuts[1:])]
            w0p.append((w8s[0][:, :, :], wt8[0, :, :, :]))
            xq = [xpiece(k) for k in range(NKB)]
            xq += [
                (x8[:, 2 * pr:2 * pr + 2, :], xt8[:, 2 * pr:2 * pr + 2, :])
                for pr in range(NPR)
            ]
            # strip-0 W pieces interleaved 1:3 with x pieces, in
            # consumption order; then bias and the remaining strips
            dma_list = []
            for i, wp in enumerate(w0p):
                dma_list.append(wp)
                dma_list += xq[3 * i:3 * i + 3]
            dma_list += xq[3 * len(w0p):]
            dma_list += [(bias_sb[:, :], biasr[:, :])]
            for s in range(1, NSTRIP):
                dma_list += [wpiece(s, 0, NKB), (w8s[s][:, :, :], wt8[s, :, :, :])]
            for di, (dst, src) in enumerate(dma_list):
                eng = nc.sync if di % 2 == 0 else nc.scalar
                eng.dma_start(dst, src)

            # ---- binarize on the DVE ----
            # bf16 k-subtiles in place; fp8 section bf16 -> fp8 out of place.
            # ranges aligned with the DMA pieces so each op only waits its
            # own piece's completion semaphore
            # bf16 section: (w >= 0) - 0.5 in place. fp8 section: the sign
            # bit is the MSB of the shipped fp8 byte (exact even where the
            # fp8 cast flushed a tiny w to +-0), so (byte < 128) - 0.5
            # yields +-0.5 with the sign of the original weight.
            def binarize(s, split_first=False):
                cuts = [c for c in ([0, 2, 5, NKB] if split_first else [0, NKB]) if c <= NKB]
                for a, b in zip(cuts, cuts[1:]):
                    if a == b:
                        continue
                    nc.vector.tensor_scalar(
                        ws[s][:, a:b, :], ws[s][:, a:b, :], 0.0, 0.5,
                        mybir.AluOpType.is_ge, mybir.AluOpType.subtract,
                    )
                nc.vector.tensor_scalar(
                    w8[s][:, :, :], w8s[s][:, :, :], 128.0, 0.5,
                    mybir.AluOpType.is_lt, mybir.AluOpType.subtract,
                )

            binarize(0, split_first=True)
            binarize(1)

            # ---- GEMM: y^T strip by strip, W stationary, x moving ----
            ev = 0

            # NOTE: evictions must stay off the Activation engine — ACTIVATE
            # instructions there drop the PE clock from 2.4GHz to ~2.0GHz
            # (measured: 16 ACTIVATE evictions -> every matmul 454ns instead
            # of 379ns, +16us total).
            def evict_block(ps_pair, g):
                """Evict one n128 block (2 psum tiles) -> bf16 out -> DRAM."""
                nonlocal ev
                ot = out_pool.tile([KB, nsh], bf16, tag="out", name=f"ot{ev}")
                for mh in range(nmh):
                    nc.vector.tensor_scalar(
                        ot[:, mh * MH:(mh + 1) * MH], ps_pair[mh][:, :],
                        2.0, bias_sb[:, g:g + 1],
                        mybir.AluOpType.mult, mybir.AluOpType.add,
                    )
                oeng = nc.sync if ev % 2 == 0 else nc.scalar
                oeng.dma_start(yt[g * KB:(g + 1) * KB, :], ot[:, :])
                ev += 1

            def strip_mms(s, bs, ps, tag):
                """Accumulate strip s over n128 blocks `bs` into psum tiles
                ps[(b_local, mh)]."""
                for k in range(NKB):
                    for bl, b in enumerate(bs):
                        lhsT = ws[s][:, k, b * KB:(b + 1) * KB]
                        for mh in range(nmh):
                            nc.tensor.matmul(
                                ps[bl * nmh + mh][:, :], lhsT, xslice(k, mh),
                                start=(k == 0), stop=False,
                            )
                for pr in range(NPR):
                    for bl, b in enumerate(bs):
                        lhsT = w8[s][:, 2 * pr:2 * pr + 2, b * KB:(b + 1) * KB]
                        for mh in range(nmh):
                            nc.tensor.matmul(
                                ps[bl * nmh + mh][:, :], lhsT,
                                x8[:, 2 * pr:2 * pr + 2, mh * MH:(mh + 1) * MH],
                                start=False, stop=(pr == NPR - 1),
                                perf_mode=mybir.MatmulPerfMode.DoubleRow,
                                skip_group_check=True,
                            )

            for s in range(NSTRIP - 1):
                ps = [
                    psum_pool.tile([KB, MH], f32, tag="ps", name=f"ps{s}_{i}")
                    for i in range(2 * nmh)
                ]
                strip_mms(s, (0, 1), ps, s)
                evict_block(ps[0:nmh], s * 2 + 0)
                evict_block(ps[nmh:2 * nmh], s * 2 + 1)
                if s + 2 < NSTRIP:
                    binarize(s + 2)

            # last strip in two 128-col halves: the first half's eviction
            # overlaps the second half's matmuls, shrinking the tail drain.
            # Evictions go half-width, split across the DVE and the
            # Activation engine (out = Identity(2*psum + bias)) so the final
            # drain is as short as possible.
            s = NSTRIP - 1
            for b in range(2):
                ps = [
                    psum_pool.tile([KB, MH], f32, tag="ps", name=f"ps{s}h{b}_{i}")
                    for i in range(nmh)
                ]
                strip_mms(s, (b,), ps, s)
                g = s * 2 + b
                ot = out_pool.tile([KB, nsh], bf16, tag="out", name=f"otl{b}")
                for mh in range(nmh):
                    nc.vector.tensor_scalar(
                        ot[:, mh * MH:(mh + 1) * MH], ps[mh][:, :],
                        2.0, bias_sb[:, g:g + 1],
                        mybir.AluOpType.mult, mybir.AluOpType.add,
                    )
                nc.sync.dma_start(yt[g * KB:(g + 1) * KB, 0:MH], ot[:, 0:MH])
                nc.scalar.dma_start(yt[g * KB:(g + 1) * KB, MH:2 * MH], ot[:, MH:2 * MH])
    nc.compile()
    return nc


def _get_nc():
    if "nc" not in _cache:
        _cache["nc"] = build_nc()
    return _cache["nc"]


def run_spmd(nc, in_maps, trace=False):
    from concourse.bass_utils import run_bass_kernel_spmd

    return run_bass_kernel_spmd(
        nc, in_maps, list(range(N_CORES)), trace=trace
    )


def pack_w(weight):
    """weight [out, in] f32 -> (wt bf16 [NSTRIP,128,NKB,SCOL],
    wt8 u8 [NSTRIP,128,2*NPR,SCOL]).

    wt carries the bf16 section (K < KB16). wt8 carries the fp8 section
    as raw fp8-e4m3 bytes (sign preserved in the MSB even when the cast
    flushes tiny weights to +-0), pair-interleaved to match DoubleRow's
    [128, 2, n] stationary layout.
    """
    wtr = weight.T                                          # [in, out] f32
    a = wtr[:KB16].astype(ml_dtypes.bfloat16)
    a = a.reshape(NKB, KB, NSTRIP, SCOL)                    # [sub, p, s, j]
    wtb = np.ascontiguousarray(a.transpose(2, 1, 0, 3))     # [s, p, sub, j]
    b = wtr[KB16:].astype(ml_dtypes.float8_e4m3).view(np.uint8)
    b = b.reshape(2 * NPR, KB, NSTRIP, SCOL)
    wt8 = np.ascontiguousarray(b.transpose(2, 1, 0, 3))
    return wtb, wt8


def _in_maps(x, weight, bias):
    x = np.asarray(x, dtype=np.float32)
    weight = np.asarray(weight, dtype=np.float32)
    bias = np.asarray(bias, dtype=np.float32)
    wtb, wt8 = pack_w(weight)
    biasr = np.ascontiguousarray(bias.reshape(D_OUT // KB, KB).T)
    maps = []
    for i in range(N_CORES):
        xs = x[i * N_SH:(i + 1) * N_SH]                    # [1024, 2048]
        xbt = xs[:, :KB16].T.astype(ml_dtypes.bfloat16)    # [1280, 1024]
        xtb = np.ascontiguousarray(
            xbt.reshape(NKB, KB, N_SH).transpose(1, 0, 2)  # [p, k, m]
        )
        x8t = xs[:, KB16:].T.astype(ml_dtypes.float8_e4m3)  # [768, 1024]
        xt8 = np.ascontiguousarray(
            x8t.reshape(2 * NPR, KB, N_SH).transpose(1, 0, 2)  # [p, sub, m]
        )
        maps.append({"xtb": xtb, "xt8": xt8, "wt": wtb, "wt8": wt8, "biasr": biasr})
    return maps


def kernel(x, weight, bias):
    nc = _get_nc()
    res = run_spmd(nc, _in_maps(x, weight, bias))
    y = np.concatenate(
        [res.results[i]["yt"].T.astype(np.float32) for i in range(N_CORES)],
        axis=0,
    )
    return np.ascontiguousarray(y)
